# revision 11
# baseline (speedup 1.0000x reference)
"""RGCN basis-decomposition message passing on 8 Trainium2 NeuronCores.

Math (matching the reference):
    seg     = edge_type * N + target
    stacked = segment_sum(edge_weights[:,None] * x[source], seg)   # (R*N, DIN)
    W       = einsum('rb,bio->rio', comb, bases)                   # (R, DIN, DOUT)
    out     = einsum('rni,rio->no', stacked.reshape(R,N,DIN), W)   # (N, DOUT)

Strategy (edge/target-parallel, output-stationary):
  * Shard targets across the 8 cores (contiguous ranges). Each core owns all
    edges pointing into its targets -> no cross-core reduction at all.
  * Host packs each core's targets into "quarters" of <=128 targets such that
    for every relation r the number of edges into (r, quarter) is <= 128.
    4 quarters = one super-tile (<=512 targets).
  * Device, per super-tile:
      - one batched indirect DMA gathers the fp16 x-rows of all edge slots
        (128 slots x 64 windows) straight from DRAM into SBUF,
      - per window (r, quarter) a DVE tensor_scalar builds a one-hot matrix
        H[slot, t_local] = (iota == t_local[slot]) * w[slot],
      - one PE matmul per window computes msgs^T @ H -> PSUM, giving the
        per-relation segment sums "stacked_T" in (DIN x t_local) orientation,
      - PSUM banks are copied (ACT/DVE) into an SBUF stacked_T tile,
      - 16 accumulating PE matmuls with W_r as stationary operand produce
        out_T(o, t) for the super-tile; result DMAs out.
  * Host reassembles: out rows are permuted back and transposed on the host.
"""

import os
import sys

sys.path.insert(0, "/opt/trn_rl_repo")

KDBG = set(os.environ.get("KDBG", "").split(","))

import numpy as np

import concourse.bacc as bacc
import concourse.bass as bass
import concourse.mybir as mybir
import concourse.tile as tile
from concourse.bass import IndirectOffsetOnAxis
from concourse.bass_utils import run_bass_kernel_spmd

N_CORES = 8
R = 16          # relations
DIN = 64
DOUT = 64
TPQ = 128       # targets per quarter
QPS = 4         # quarters per super-tile
TPS = TPQ * QPS # targets per super-tile (512)
WPS = R * QPS   # windows per super-tile (64)
SLOTS = 128     # edge slots per window (= matmul K)
GATHER_SPLIT = 2  # indirect-gather calls per super-tile (must divide WPS)

f32 = mybir.dt.float32
f16 = mybir.dt.float16
i16 = mybir.dt.int16
i32 = mybir.dt.int32


# ----------------------------------------------------------------------------
# Host-side packing
# ----------------------------------------------------------------------------

def _pack_core(tgts, src_e, tgt_e, rel_e, w_e, n_sups):
    """Pack one core's targets into quarters and its edges into window slots.

    tgts:   (Tc,) global target ids owned by this core
    *_e:    this core's edges (target in tgts)
    Returns (offs, segl, wgt, perm) with shapes
      offs (S,128,WPS) i32, segl (S,128,WPS) i16, wgt (S,128,WPS) f16,
      perm (S, TPS) i64   (global target id per out column, -1 = unused)
    """
    Tc = len(tgts)
    # per-target per-relation degree (only edges of this core)
    loc = {t: i for i, t in enumerate(tgts)}
    tloc_e = np.searchsorted(tgts, tgt_e)  # tgts sorted ascending
    assert np.array_equal(tgts[tloc_e], tgt_e)
    deg = np.zeros((Tc, R), dtype=np.int64)
    np.add.at(deg, (tloc_e, rel_e), 1)

    # greedy quarter packing: fill quarters in target order, close a quarter
    # when it has 128 targets or any relation would exceed 128 edges
    q_of_t = np.empty(Tc, dtype=np.int64)
    tl_of_t = np.empty(Tc, dtype=np.int64)
    q_idx = 0
    q_cnt = np.zeros(R, dtype=np.int64)
    q_n = 0
    for i in range(Tc):
        d = deg[i]
        if q_n == TPQ or np.any(q_cnt + d > SLOTS):
            q_idx += 1
            q_cnt = np.zeros(R, dtype=np.int64)
            q_n = 0
        q_of_t[i] = q_idx
        tl_of_t[i] = q_n
        q_cnt += d
        q_n += 1
    n_quarters = q_idx + 1
    S = (n_quarters + QPS - 1) // QPS
    assert n_sups is None or S <= n_sups, (S, n_sups)
    if n_sups is not None:
        S = n_sups

    offs = np.zeros((S, SLOTS, WPS), dtype=np.int32)
    segl = np.full((S, SLOTS, WPS), -1, dtype=np.float32)
    wgt = np.zeros((S, SLOTS, WPS), dtype=np.float32)
    perm = np.full((S, TPS), -1, dtype=np.int64)

    sup_of_t = q_of_t // QPS
    qin_of_t = q_of_t % QPS
    perm[sup_of_t, qin_of_t * TPQ + tl_of_t] = tgts

    # edge -> (sup, window=r*QPS+q, slot)
    sup_e = sup_of_t[tloc_e]
    win_e = rel_e * QPS + qin_of_t[tloc_e]
    key = sup_e * WPS + win_e
    order = np.argsort(key, kind="stable")
    sk = key[order]
    first = np.searchsorted(sk, sk, side="left")
    slot_sorted = np.arange(len(order)) - first
    assert slot_sorted.size == 0 or slot_sorted.max() < SLOTS
    slot_e = np.empty_like(slot_sorted)
    slot_e[order] = slot_sorted

    offs[sup_e, slot_e, win_e] = src_e.astype(np.int32)
    segl[sup_e, slot_e, win_e] = tl_of_t[tloc_e].astype(np.float32)
    wgt[sup_e, slot_e, win_e] = w_e.astype(np.float32)
    return offs, segl, wgt, perm


def _pack_all(x_n, source, target, edge_type, edge_weights):
    N = x_n.shape[0]
    bounds = np.linspace(0, N, N_CORES + 1).astype(np.int64)
    core_of_t = np.searchsorted(bounds, np.arange(N), side="right") - 1
    core_e = core_of_t[target]

    packed = []
    for c in range(N_CORES):
        m = core_e == c
        tgts = np.arange(bounds[c], bounds[c + 1])
        packed.append((tgts, source[m], target[m], edge_type[m], edge_weights[m]))

    # first pass to learn per-core S, then repack padded to the max
    n_sups = []
    for tgts, s, t, r, w in packed:
        tmp = _pack_core(tgts, s, t, r, w, None)
        n_sups.append(tmp[0].shape[0])
    S = max(n_sups)
    cores = [
        _pack_core(tgts, s, t, r, w, S) for (tgts, s, t, r, w) in packed
    ]
    return S, cores


# ----------------------------------------------------------------------------
# Device program
# ----------------------------------------------------------------------------

def _build_program(N, S):
    nc = bacc.Bacc(
        "TRN2",
        target_bir_lowering=False,
        debug=False,
        enable_asserts=False,
        num_devices=N_CORES,
    )
    x16 = nc.dram_tensor("x16", [N, DIN], f16, kind="ExternalInput").ap()
    offs = nc.dram_tensor("offs", [S, SLOTS, WPS], i32, kind="ExternalInput").ap()
    segl = nc.dram_tensor("segl", [S, SLOTS, WPS], f32, kind="ExternalInput").ap()
    wgt = nc.dram_tensor("wgt", [S, SLOTS, WPS], f32, kind="ExternalInput").ap()
    wmat = nc.dram_tensor("wmat", [DIN, R * DOUT], f32, kind="ExternalInput").ap()
    out = nc.dram_tensor("out", [S, DOUT, TPS], f32, kind="ExternalOutput").ap()

    with tile.TileContext(nc) as tc:
        with (
            tc.tile_pool(name="const", bufs=1) as cpool,
            tc.tile_pool(name="meta", bufs=3) as mpool,
            tc.tile_pool(name="msgs", bufs=3) as gpool,
            tc.tile_pool(name="hmat", bufs=8) as hpool,
            tc.tile_pool(name="stk", bufs=2) as spool,
            tc.tile_pool(name="osb", bufs=2) as opool,
            tc.tile_pool(name="ps1", bufs=4, space="PSUM") as ps1,
            tc.tile_pool(name="ps2", bufs=2, space="PSUM") as ps2,
        ):
            iota_t = cpool.tile([128, TPQ], i16)
            nc.gpsimd.iota(iota_t[:], pattern=[[1, TPQ]], base=0, channel_multiplier=0)
            # W replicated into both partition halves so phase-2 lhsT can
            # share the base partition of either stacked_T half (HW requires
            # lhsT.base_partition == rhs.base_partition).
            wm_sb = cpool.tile([2 * DIN, R * DOUT], f32)
            nc.sync.dma_start(out=wm_sb[:DIN, :], in_=wmat[:, :])
            nc.sync.dma_start(out=wm_sb[DIN:, :], in_=wmat[:, :])

            for s in range(S):
                off_sb = mpool.tile([SLOTS, WPS], i32, tag="off")
                sg_sb = mpool.tile([SLOTS, WPS], f32, tag="sg")
                wg_sb = mpool.tile([SLOTS, WPS], f32, tag="wg")
                nc.sync.dma_start(out=off_sb[:], in_=offs[s, :, :])
                nc.sync.dma_start(out=sg_sb[:], in_=segl[s, :, :])
                nc.sync.dma_start(out=wg_sb[:], in_=wgt[s, :, :])

                msgs = gpool.tile([SLOTS, WPS, DIN], f16, tag="msgs")
                # HW limitation: indirect DMA honours ONE offset per
                # partition, so gather one 128-row window per call.
                if "nogather" in KDBG:
                    nc.gpsimd.memset(msgs[:], 0)
                else:
                    for w in range(WPS):
                        nc.gpsimd.indirect_dma_start(
                            out=msgs[:, w, :],
                            out_offset=None,
                            in_=x16[:, :],
                            in_offset=IndirectOffsetOnAxis(
                                ap=off_sb[:, w : w + 1], axis=0
                            ),
                        )

                stk = spool.tile([128, (R // 2) * TPS], f32, tag="stk")
                for rp in range(R // 2):
                    if "nomm1" in KDBG:
                        break
                    ps = ps1.tile([128, TPS], f32)
                    for half in range(2):
                        r = rp * 2 + half
                        for q in range(QPS):
                            w = r * QPS + q
                            hm = hpool.tile([SLOTS, TPQ], f16, tag="hm")
                            nc.vector.tensor_scalar(
                                out=hm[:],
                                in0=iota_t[:],
                                scalar1=sg_sb[:, w : w + 1],
                                scalar2=wg_sb[:, w : w + 1],
                                op0=mybir.AluOpType.is_equal,
                                op1=mybir.AluOpType.mult,
                            )
                            if "nomm1" not in KDBG:
                                nc.tensor.matmul(
                                    ps[half * DIN : (half + 1) * DIN, q * TPQ : (q + 1) * TPQ],
                                    lhsT=msgs[:, w, :],
                                    rhs=hm[:],
                                    start=True,
                                    stop=True,
                                )
                    if rp % 2 == 0:
                        nc.scalar.copy(stk[:, rp * TPS : (rp + 1) * TPS], ps[:])
                    else:
                        nc.vector.tensor_copy(stk[:, rp * TPS : (rp + 1) * TPS], ps[:])

                if "nomm1" in KDBG:
                    nc.gpsimd.memset(stk[:], 0)
                osb = opool.tile([DOUT, TPS], f32, tag="osb")
                if "noph2" in KDBG:
                    nc.vector.tensor_copy(osb[:], stk[0:DOUT, 0:TPS])
                else:
                    # HW: an accumulation group must keep a fixed operand
                    # base partition -> one group per stacked half, then
                    # copy + add to combine.
                    psA = ps2.tile([DOUT, TPS], f32, tag="psA")
                    psB = ps2.tile([DOUT, TPS], f32, tag="psB")
                    for half, pst in ((0, psA), (1, psB)):
                        rs = list(range(half, R, 2))
                        for i, r in enumerate(rs):
                            nc.tensor.matmul(
                                pst[:],
                                lhsT=wm_sb[
                                    half * DIN : (half + 1) * DIN,
                                    r * DOUT : (r + 1) * DOUT,
                                ],
                                rhs=stk[
                                    half * DIN : (half + 1) * DIN,
                                    (r // 2) * TPS : (r // 2 + 1) * TPS,
                                ],
                                start=(i == 0),
                                stop=(i == len(rs) - 1),
                            )
                    nc.scalar.copy(osb[:], psA[:])
                    nc.vector.tensor_add(osb[:], osb[:], psB[:])
                nc.sync.dma_start(out=out[s, :, :], in_=osb[:])

    nc.compile()
    return nc


# ----------------------------------------------------------------------------
# Entry point
# ----------------------------------------------------------------------------

def kernel(x, source, target, edge_type, edge_weights, bases, comb):
    x = np.asarray(x, dtype=np.float32)
    source = np.asarray(source).astype(np.int64)
    target = np.asarray(target).astype(np.int64)
    edge_type = np.asarray(edge_type).astype(np.int64)
    edge_weights = np.asarray(edge_weights, dtype=np.float32)
    bases = np.asarray(bases, dtype=np.float32)
    comb = np.asarray(comb, dtype=np.float32)

    N = x.shape[0]
    S, cores = _pack_all(x, source, target, edge_type, edge_weights)

    x16 = np.ascontiguousarray(x.astype(np.float16))
    W = np.einsum("rb,bio->rio", comb, bases).astype(np.float32)
    wmat = np.ascontiguousarray(W.transpose(1, 0, 2).reshape(DIN, R * DOUT))

    nc = _build_program(N, S)
    in_maps = []
    for offs_c, segl_c, wgt_c, _perm in cores:
        in_maps.append(
            {"x16": x16, "offs": offs_c, "segl": segl_c, "wgt": wgt_c, "wmat": wmat}
        )
    res = run_bass_kernel_spmd(nc, in_maps, core_ids=list(range(N_CORES)))

    out_full = np.zeros((N, DOUT), dtype=np.float32)
    for c in range(N_CORES):
        o = res.results[c]["out"]  # (S, DOUT, TPS)
        perm = cores[c][3]  # (S, TPS)
        cols = o.transpose(0, 2, 1).reshape(-1, DOUT)  # (S*TPS, DOUT)
        pf = perm.reshape(-1)
        m = pf >= 0
        out_full[pf[m]] = cols[m]
    return out_full
